# revision 32
# baseline (speedup 1.0000x reference)
"""Trainium2 Bass kernel for nn_ConstraintsModule (fuzzy-logic constraint
propagation).

Algorithm notes
---------------
The reference computes, twice (apply-1 with active=full_body, apply-2 with
active=unsat_head and goal-masked bodies):

    body_rev[b,c,a] = pb[c,a] + v[b,a]*(nb-pb)      -> max over a
    body_min[b,c]   = active[b,c] * (1 - max_a body_rev)
    lb[b,n] = max_c body_min * pos_head[c,n] ; ub = 1 - max_c body_min*neg_head
    u = max(min(lb,ub), min(max(lb,ub), v))  ==  median3(lb, ub, v)

Key identities (z = per-literal body_rev value, y = 1-z, activity gates
folded in as extra/merged table rows -- exact because goals are binary):

    bm = body_min = min_k y_k      y_pos1 = min(p,g), y_neg1 = min(1-p,1-g)
    W  = 1 - bm   = max_k z_k      apply-2 rows: u1 / 1-u1 / const, plus one
                                   head row (unsat indicator)

Precision: the harness checks POINTWISE RELATIVE error, so small outputs
must stay relative-precise through bf16.  Small x and 1-x cannot both be
kept in one bf16 number, so everything flows in BOTH domains at once:

  - merged table [128 slots, R rows, 2B] bf16 with columns [y | -z];
    min(-z) = -max(z), so ONE min-tree yields mv = [bm | -W].
    The table is row-split (8+4) across the two HWDGE rings so the first
    subtree overlaps the second transfer.
  - mvneg = [W | -bm] (two 4x-mode negate-copies of mv's halves).
  - 6 one-hot scatter matmuls, 256-row moving:
      pos layers (moving mv):    psum = [oneh@bm | -oneh@W],  MAX-combine
        -> posT = [lb | -lbc]
      neg layers (moving mvneg): psum = [oneh@W  | -oneh@bm], MIN-combine
        -> negT = [ub | -ubc]
    The ones-slot (y=0 / -z=-1) absorbs empty (layer, atom) positions and
    is neutral for every half/combine.  One-hot selections in f32 PSUM are
    exact, and every post-PSUM value is an already-bf16 number, so the
    whole back half runs bf16 (2x DVE) with zero extra error.
  - [u | -uc] = median3(posT, negT, [base | -basec]) -- median is odd and
    elementwise, so one 4-op wide chain serves both domains.  The host
    flips the sign of the right half; apply-2 tables pick u1 (small) or
    u1c = 1-u1 (small) per row so no catastrophic cancellation happens.

A line of ~15 dummy matmuls on a zeroed tile keeps the PE busy from the
start so it reaches full p-state before the real matmuls issue.

Sharding: constraints are owned by the core that owns their head atom
(atom range of 128 per core) so the head-scatter and clamp are core-local.
One compiled SPMD program serves both applies; the host gathers per-literal
value rows between launches (pure data layout, not device time).
"""
import numpy as np

import concourse.bass as bass
import concourse.tile as tile
from concourse import mybir
from concourse.tile import ScopedClock
from concourse.bass_utils import run_bass_kernel_spmd

B = 128
NCOL = 2048
NA = 1024
C = 512
NCORES = 8
SLOTS = 128          # constraint slots per core (padded)
NLOC = 128           # atoms per core
ONES_SLOT = SLOTS - 1
N_WARMUP = 11        # dummy PE matmuls ramping the p-state


class FixedTileContext(tile.TileContext):
    """Two workarounds for this walrus/NRT combo: (1) skip the tail
    clear_and_free_semaphores — its InstSemClear makes NRT reject the NEFF at
    load, and NRT resets semaphores per execution anyway; (2) multi-wait
    instructions are split afterwards by split_multi_waits()."""

    def _drain_and_barrier(self, tick_clock, wait_clock):
        drain_inst = self.nc.sync.drain()
        wait_clock.add_sem_waits(
            drain_inst.ins, ScopedClock({None: tick_clock.global_clock})
        )
        self.nc.all_engine_barrier()
        assert self.sems is not None
        popped = self.nc._tile_sem_poison_stack.pop()
        assert popped is self._sem_poison
        self.nc.all_engine_barrier()


def split_multi_waits(nc: bass.Bass) -> int:
    """walrus here accepts only ONE sync wait per instruction; Tile's
    add_semaphores attaches several.  Hoist all but one wait onto fresh
    same-engine nops placed immediately before the instruction (engine
    program order is preserved, so blocking semantics are identical)."""
    n_split = 0
    for f in nc.m.functions:
        for b in f.blocks:
            new = []
            for ins in b.instructions:
                si = ins.sync_info
                waits = list(si.on_wait) if si and si.on_wait else []
                if len(waits) > 1:
                    for w in waits[:-1]:
                        nop = mybir.InstNoOp(
                            name=f"waitsplit-{n_split}", ins=[], outs=[])
                        n_split += 1
                        nop.engine = ins.engine
                        nop.sync_info = mybir.SyncInfo(on_wait=[w], on_update=[])
                        new.append(nop)
                    ins.sync_info = mybir.SyncInfo(
                        on_wait=[waits[-1]],
                        on_update=list(si.on_update) if si.on_update else [])
                new.append(ins)
            b.instructions = new
    return n_split


_PROGRAM_CACHE = {}
SPLIT_WAITS = True  # set False when running under CoreSim (sim chokes on the
                    # synthesized nops, and doesn't need the split anyway)


def _tree_rows(n: int) -> int:
    """Smallest r >= n whose halving path only hits odd values when the
    leftover fold is a single row (r in {1,2,3} or 2*allowed)."""
    def ok(r):
        while r > 1:
            if r % 2 and r != 3:
                return False
            r //= 2
        return True
    while not ok(n):
        n += 1
    return n


def _row_split(rows: int) -> tuple:
    """(ra, rb) tree-able halves: equal split keeps both subtree level
    shapes identical so the two trees can share (and WAR-serialize on)
    the same level buffers."""
    ra = _tree_rows((rows + 1) // 2)
    rb = _tree_rows(max(rows - (rows + 1) // 2, 1))
    return max(ra, rb), max(ra, rb)


def _min_tree(nc, pool, cur, n, width, shared, name):
    """Elementwise-min tree over the row axis of cur [128, n, width] (bf16,
    2x DVE mode).  Intermediate levels use `shared` tags (so two subtrees
    WAR-serialize into [first..., second...] engine order); the final level
    gets a unique `name` tile.  Returns a [128, 1, width] AP."""
    bf16 = mybir.dt.bfloat16
    mn = mybir.AluOpType.min
    lvl = 0
    while n > 1:
        h = n // 2
        is_final = (h == 1) and (n % 2 == 0)
        tag = f"{name}f" if is_final else f"{shared}{lvl}"
        nxt = pool.tile([SLOTS, h, width], bf16, tag=tag, name=tag)
        nc.vector.tensor_tensor(
            nxt[:, 0:h, :], cur[:, 0:h, :], cur[:, h:2 * h, :], mn)
        if n % 2:
            assert h == 1, "pad rows with _tree_rows on the host"
            nx2 = pool.tile([SLOTS, 1, width], bf16,
                            tag=f"{name}f", name=f"{name}f")
            nc.vector.tensor_tensor(
                nx2[:, 0:1, :], nxt[:, 0:1, :], cur[:, n - 1:n, :], mn)
            nxt = nx2
        cur, n = nxt, h
        lvl += 1
    return cur


def _build_program(ra: int, rb: int, lpos: int, lneg: int) -> bass.Bass:
    """One SPMD apply phase.  Inputs are per-core; the same program serves
    both applies (taba / tabb / pack inputs differ per launch)."""
    key = (ra, rb, lpos, lneg)
    if key in _PROGRAM_CACHE:
        return _PROGRAM_CACHE[key]

    f32, bf16 = mybir.dt.float32, mybir.dt.bfloat16
    mn, mx = mybir.AluOpType.min, mybir.AluOpType.max
    W2 = 2 * B
    L = lpos + lneg
    nc = bass.Bass(num_devices=NCORES)
    taba_d = nc.declare_dram_parameter("taba", [SLOTS, ra * W2], bf16, isOutput=False)
    tabb_d = nc.declare_dram_parameter("tabb", [SLOTS, rb * W2], bf16, isOutput=False)
    # pack = [scat layers (L x NLOC) | base | -basec] per partition
    pack_d = nc.declare_dram_parameter(
        "pack", [SLOTS, (L + 2) * NLOC], bf16, isOutput=False)
    u_d = nc.declare_dram_parameter("u", [NLOC, W2], bf16, isOutput=True)

    with FixedTileContext(nc) as tc:
        with (
            tc.tile_pool(name="sbuf", bufs=1) as pool,
            tc.tile_pool(name="psum", bufs=1, space="PSUM") as psum,
        ):
            # PE p-state warmup: dummy matmuls on a zeroed tile from t~1us
            zeros = pool.tile([SLOTS, 512], bf16)
            nc.gpsimd.memset(zeros[:], 0.0)
            pwarm = psum.tile([SLOTS, 512], f32)
            for i in range(N_WARMUP):
                nc.tensor.matmul(pwarm[:], zeros[:, 0:128], zeros[:],
                                 start=True, stop=True)

            out = pool.tile([NLOC, W2], bf16)

            taba = pool.tile([SLOTS, ra, W2], bf16)
            nc.sync.dma_start(taba[:], taba_d[:].rearrange("p (k w) -> p k w", k=ra))
            tabb = pool.tile([SLOTS, rb, W2], bf16)
            nc.scalar.dma_start(tabb[:], tabb_d[:].rearrange("p (k w) -> p k w", k=rb))
            # pack rides the scalar ring behind tabb: its transfer then cannot
            # preempt the table transfers on the shared DMA engines
            pack = pool.tile([SLOTS, (L + 2) * NLOC], bf16)
            nc.scalar.dma_start(pack[:], pack_d[:])

            # both subtrees share level buffers (same tag): the WAR hazard
            # pins the engine order to [suba..., subb...] so suba's levels
            # are not head-of-line blocked behind subb's late DMA
            suba = _min_tree(nc, pool, taba[:], ra, W2, "t", "ta")
            subb = _min_tree(nc, pool, tabb[:], rb, W2, "t", "tb")
            mv = pool.tile([SLOTS, W2], bf16)
            nc.vector.tensor_tensor(
                mv[:].rearrange("p (o w) -> p o w", o=1),
                suba[:], subb[:], mn)
            # mvneg = [W | -bm] so the neg group can MIN-combine directly
            mvneg = pool.tile([SLOTS, W2], bf16)
            nc.vector.tensor_scalar(
                mvneg[:, 0:B], mv[:, B:W2], -1.0, None, mybir.AluOpType.mult)
            nc.vector.tensor_scalar(
                mvneg[:, B:W2], mv[:, 0:B], -1.0, None, mybir.AluOpType.mult)

            pb = []
            for l in range(L):
                pt = psum.tile([NLOC, W2], f32, tag=f"pb{l}", name=f"pb{l}")
                nc.tensor.matmul(
                    pt[:], pack[:, l * NLOC:(l + 1) * NLOC],
                    mv[:] if l < lpos else mvneg[:],
                    start=True, stop=True)
                pb.append(pt)

            def combine(l0, nlayers, comb, name):
                acc = pool.tile([NLOC, W2], bf16, tag=f"{name}0")
                nc.scalar.copy(acc[:], pb[l0][:])
                for i in range(1, nlayers):
                    nxt = pool.tile([NLOC, W2], bf16, tag=f"{name}{i}")
                    nc.vector.tensor_tensor(nxt[:], acc[:], pb[l0 + i][:], comb)
                    acc = nxt
                return acc

            posT = combine(0, lpos, mx, "pos")       # [lb | -lbc]
            negT = combine(lpos, lneg, mn, "neg")    # [ub | -ubc]

            # [u | -uc] = median3(posT, negT, [base | -basec]), elementwise
            cv = pack[:, L * NLOC:(L + 2) * NLOC]
            lo = pool.tile([NLOC, W2], bf16)
            nc.vector.tensor_tensor(lo[:], posT[:], negT[:], mn)
            hi = pool.tile([NLOC, W2], bf16)
            nc.vector.tensor_tensor(hi[:], posT[:], negT[:], mx)
            mid = pool.tile([NLOC, W2], bf16)
            nc.vector.tensor_tensor(mid[:], hi[:], cv, mn)
            nc.vector.tensor_tensor(out[:], lo[:], mid[:], mx)
            nc.sync.dma_start(u_d[:], out[:])

    if SPLIT_WAITS:
        split_multi_waits(nc)
    _PROGRAM_CACHE[key] = nc
    return nc


class _Prep:
    """Host-side preprocessing: everything that doesn't depend on the
    intermediate u1, plus the static literal->table-row maps."""

    def __init__(self, preds, goal, atoms, pos_body, neg_body, pos_head, neg_head):
        f32 = np.float32
        import ml_dtypes
        self.bf16 = ml_dtypes.bfloat16
        self.atoms = np.asarray(atoms)
        self.p = preds[:, self.atoms].astype(f32)            # [B, NA]
        self.g = goal[:, self.atoms].astype(f32)
        self.pT = np.ascontiguousarray(self.p.T)             # [NA, B]
        self.gT = np.ascontiguousarray(self.g.T)

        hsum = pos_head + neg_head
        assert np.all(hsum.sum(axis=1) == 1.0), "heads must be one-hot"
        self.h = np.argmax(hsum, axis=1)                     # [C]
        self.head_is_pos = pos_head[np.arange(C), self.h] == 1.0
        owner = self.h // NLOC

        pos_lists = [np.nonzero(pos_body[c])[0] for c in range(C)]
        neg_lists = [np.nonzero(neg_body[c])[0] for c in range(C)]
        ncnt = np.array([len(pos_lists[c]) + len(neg_lists[c]) for c in range(C)])
        rows = int(ncnt.max()) + 1           # +1 head row (apply-2)
        self.ra, self.rb = _row_split(rows)
        self.rows = self.ra + self.rb

        ROW_CONST = 2 * NA            # y=1 / -z=0 padding row
        ROW_ONES = 2 * NA + 1 + C     # y=0 / -z=-1 row for the ones-slot
        self.nvrows = 2 * NA + 2 + C

        self.cons = []        # per core: constraint ids in slot order
        self.rmap = []        # per core: [SLOTS, rows] int ids into vcat
        lpos_need, lneg_need = 1, 1
        layer_asn = []        # per core: (slot, is_pos, layer, nloc)
        for i in range(NCORES):
            ci = np.nonzero(owner == i)[0]
            assert len(ci) < SLOTS, f"core {i}: need a free ones-slot"
            self.cons.append(ci)
            rm = np.full((SLOTS, self.rows), ROW_CONST, dtype=np.int64)
            rm[ONES_SLOT, :] = ROW_ONES
            for s, c in enumerate(ci):
                rr = np.concatenate([pos_lists[c], NA + neg_lists[c]])
                rm[s, : len(rr)] = rr
                rm[s, ncnt[c]] = 2 * NA + 1 + c      # head row (apply-2)
            self.rmap.append(rm)

            counts = {}
            asn = []
            for s, c in enumerate(ci):
                key = (self.h[c] % NLOC, bool(self.head_is_pos[c]))
                l = counts.get(key, 0)
                counts[key] = l + 1
                asn.append((s, key[1], l, key[0]))
                if key[1]:
                    lpos_need = max(lpos_need, l + 1)
                else:
                    lneg_need = max(lneg_need, l + 1)
            layer_asn.append(asn)

        self.lpos, self.lneg = lpos_need, lneg_need
        L = self.lpos + self.lneg

        self.scat = []
        for i in range(NCORES):
            sc = np.zeros((SLOTS, L, NLOC), dtype=f32)
            for s, is_pos, l, n in layer_asn[i]:
                li = l if is_pos else self.lpos + l
                sc[s, li, n] = 1.0
            # point every empty (layer, atom) at the ones-slot: neutral for
            # both halves of both combine kinds
            for l in range(L):
                empty = sc[:, l, :].sum(axis=0) == 0.0
                sc[ONES_SLOT, l, empty] = 1.0
            self.scat.append(sc.astype(self.bf16))

    def tables(self, yv: np.ndarray, zv: np.ndarray):
        """yv/zv: [nvrows, B] f32 -> per-core bf16 (taba, tabb) halves of the
        merged [SLOTS, rows, 2B] = [y | -z] table."""
        vv = np.concatenate([yv, -zv], axis=1)   # [nvrows, 2B]
        v16 = vv.astype(self.bf16)
        out = []
        for i in range(NCORES):
            t = v16[self.rmap[i]]                # [SLOTS, rows, 2B]
            out.append((np.ascontiguousarray(t[:, :self.ra].reshape(SLOTS, -1)),
                        np.ascontiguousarray(t[:, self.ra:].reshape(SLOTS, -1))))
        return out


def kernel(preds, goal, atoms, pos_body, neg_body, pos_head, neg_head):
    preds = np.asarray(preds)
    prep = _Prep(np.asarray(preds, np.float32), np.asarray(goal, np.float32),
                 atoms, np.asarray(pos_body, np.float32),
                 np.asarray(neg_body, np.float32),
                 np.asarray(pos_head, np.float32),
                 np.asarray(neg_head, np.float32))
    nc = _build_program(prep.ra, prep.rb, prep.lpos, prep.lneg)
    core_ids = list(range(NCORES))
    f32 = np.float32
    L = prep.lpos + prep.lneg

    def launch(yv, zv, baseT, basecT):
        tabs = prep.tables(yv, zv)
        in_maps = []
        for i in range(NCORES):
            bsl = baseT[i * NLOC:(i + 1) * NLOC].astype(prep.bf16)
            bscl = (-basecT[i * NLOC:(i + 1) * NLOC]).astype(prep.bf16)
            pack = np.concatenate(
                [prep.scat[i].reshape(SLOTS, L * NLOC), bsl, bscl], axis=1)
            in_maps.append({
                "taba": tabs[i][0],
                "tabb": tabs[i][1],
                "pack": np.ascontiguousarray(pack),
            })
        res = run_bass_kernel_spmd(nc, in_maps, core_ids)
        full = np.concatenate(
            [np.asarray(res.results[i]["u"], dtype=f32)
             for i in range(NCORES)], axis=0)                # [NA, 2B]
        return full[:, :B], -full[:, B:]

    pT, gT, h = prep.pT, prep.gT, prep.h
    ones = np.ones((1, B), f32)
    zeros = np.zeros((1, B), f32)

    # apply 1: y_pos = min(p,g), y_neg = min(1-p,1-g); head rows neutral
    yv1 = np.concatenate([
        np.minimum(pT, gT), np.minimum(1.0 - pT, 1.0 - gT),
        ones, np.ones((C, B), f32), zeros], axis=0)
    zv1 = np.concatenate([
        np.maximum(1.0 - pT, 1.0 - gT), np.maximum(pT, gT),
        zeros, np.zeros((C, B), f32), ones], axis=0)
    u1T, u1cT = launch(yv1, zv1, pT, 1.0 - pT)

    # apply 2: y_pos = g?1:u1, y_neg = g?1-u1:1 with 1-u1 := u1c;
    # head row y = unsat = pos_head ? 1-g[h] : g[h]  (exact 0/1)
    gh = gT[h]                                               # [C, B]
    hy = np.where(prep.head_is_pos[:, None], 1.0 - gh, gh)
    yv2 = np.concatenate([
        np.where(gT == 1.0, 1.0, u1T),
        np.where(gT == 1.0, u1cT, 1.0),
        ones, hy, zeros], axis=0).astype(f32)
    zv2 = np.concatenate([
        np.where(gT == 1.0, 0.0, u1cT),
        np.where(gT == 1.0, u1T, 0.0),
        zeros, 1.0 - hy, ones], axis=0).astype(f32)
    u2T, _ = launch(yv2, zv2, u1T, u1cT)

    out = np.array(preds, dtype=preds.dtype, copy=True)
    out[:, prep.atoms] = u2T.T.astype(preds.dtype)
    return out
